# revision 13
# baseline (speedup 1.0000x reference)
"""Embedding lookup (weight[indices]) on 8 TRN2 NeuronCores.

Strategy: replicate the 1M x 128 table in each core's HBM, cast to bf16 on
the host (rel err of the bf16 round trip is <= 2^-9, far inside the 2e-2
gate, and it halves both gather-read and store-write HBM traffic).  Shard
the 4096*200 = 819200 indices 8 ways (data parallel).

The gather primitive is nc.gpsimd.dma_gather (InstDMAGatherAnt): one op
generates thousands of 256 B descriptors, vs indirect_dma_start which on
real HW honors only ONE offset per partition (128 descriptors/op, ~1 us
fixed SWDGE cost each -- the v0 kernel's bottleneck at 800 ops/core).

dma_gather takes int16 indices, so the host buckets each core's indices
by 32768-row table chunk (31 chunks cover 1M rows), packs each bucket
0-padded to a fixed CAP, wraps indices into 16 partitions (slot i ->
[i%16, i//16]) and replicates them across the 8 Q7 cores' partition
groups (required by the ucode -- verified on HW).  One dma_gather op is
limited by the SWDGE descriptor-ring carveout (num_idxs ~2k crashes the
device -- found by HW bisect), so each bucket is gathered in NI-sized
sub-ops into one SBUF tile, then stored contiguously; the host applies
the inverse permutation (sub-op j, slot r -> partition r%128, group
j*(NI/128)+r//128; the store puts (p, g) at out row c*CAP + p*G + g).
"""

import numpy as np
import ml_dtypes

NUM_EMB = 1_000_000
D = 128
N_CORES = 8
P = 128

CHUNK = 32768                     # int16-addressable table chunk
N_BUCKETS = -(-NUM_EMB // CHUNK)  # 31
# Per-bucket capacity.  Bucket counts are Binomial(102400, 32768/1e6):
# mean 3355, std 57; CAP = 3840 is mean + 8.5 sigma (overflow probability
# ~1e-17 per run).  Must be a multiple of NI.
CAP = 3840
G = CAP // 128

# tuning knobs
# With single_packet=True, max num_idxs per dma_gather op is 1024 (the
# 64-descriptor-per-engine packet ceiling; 1280+ crashes the device --
# found by HW bisect).  single_packet=False lifts the cap.
SUBS = [1024, 1024, 1024, 768]  # per-bucket sub-op sizes, sum == CAP, each % 128 == 0
SP = True          # single_packet: True caps num_idxs at 1024 (64-desc packet limit)
NQ = 4             # SWDGE queues (ucode max 4): separate desc rings + Q7 pairs
BUFS = 4           # SBUF data-tile buffering depth
TABLE_DT = "bf16"  # "bf16" or "f32"

_CACHE = {}


def _build_bass(per_core: int, cap: int, subs: tuple, bufs: int, dt_name: str,
                reps: int = 1, nq: int = NQ, sp: bool = None):
    if sp is None:
        sp = SP
    import concourse.bacc as bacc
    import concourse.mybir as mybir
    import concourse.tile as tile

    key = (per_core, cap, tuple(subs), bufs, dt_name, reps, nq, sp)
    if key in _CACHE:
        return _CACHE[key]

    dt = mybir.dt.bfloat16 if dt_name == "bf16" else mybir.dt.float32
    g = cap // 128
    assert sum(subs) == cap and all(ni % 128 == 0 for ni in subs)

    nc = bacc.Bacc(
        "TRN2",
        target_bir_lowering=False,
        debug=False,
        num_devices=N_CORES,
        num_swdge_queues=nq,
    )
    idx16 = nc.dram_tensor(
        "idx16", [P, N_BUCKETS * (cap // 16)], mybir.dt.int16, kind="ExternalInput"
    )
    weight = nc.dram_tensor("weight", [NUM_EMB, D], dt, kind="ExternalInput")
    out = nc.dram_tensor("out", [N_BUCKETS * cap, D], dt, kind="ExternalOutput")

    with tile.TileContext(nc) as tc:
        with (
            tc.tile_pool(name="idxp", bufs=1) as idxp,
            tc.tile_pool(name="data", bufs=bufs) as datap,
        ):
            idx_tile = idxp.tile([P, N_BUCKETS * (cap // 16)], mybir.dt.int16)
            nc.sync.dma_start(idx_tile[:], idx16[:])

            def body():
                op = 0
                for c in range(N_BUCKETS):
                    rows = min(CHUNK, NUM_EMB - c * CHUNK)
                    dtile = datap.tile([P, g * D], dt)
                    goff = 0
                    for j, ni in enumerate(subs):
                        gsub = ni // 128
                        base = c * (cap // 16) + sum(subs[:j]) // 16
                        nc.gpsimd.dma_gather(
                            out_ap=dtile[
                                :, goff * D : (goff + gsub) * D
                            ].rearrange("p (g e) -> p g e", g=gsub),
                            in_ap=weight[c * CHUNK : c * CHUNK + rows, :],
                            idxs_ap=idx_tile[:, base : base + ni // 16],
                            num_idxs=ni,
                            num_idxs_reg=ni,
                            elem_size=D,
                            queue_num=op % nq,
                            single_packet=sp,
                        )
                        goff += gsub
                        op += 1
                    nc.sync.dma_start(
                        out[c * cap : (c + 1) * cap, :].rearrange(
                            "(p g) e -> p (g e)", p=P
                        ),
                        dtile[:],
                    )

            if reps == 1:
                body()
            else:
                with tc.For_i(0, reps, 1):
                    body()
    nc.compile()
    _CACHE[key] = nc
    return nc


def _host_pack(idx_core: np.ndarray):
    """Bucket one core's indices by table chunk.

    Returns (idx16 [P, N_BUCKETS*CAP/16] int16 for the device,
             dev_row [n] int64 mapping original position -> device out row)."""
    n = idx_core.shape[0]
    c = idx_core >> 15
    loc = (idx_core & 32767).astype(np.int16)
    order = np.argsort(c, kind="stable")
    counts = np.bincount(c, minlength=N_BUCKETS)
    if counts.max() > CAP:
        raise RuntimeError(f"bucket overflow: {counts.max()} > CAP={CAP}")
    starts = np.zeros(N_BUCKETS, np.int64)
    np.cumsum(counts[:-1], out=starts[1:])
    c_sorted = c[order]
    ranks = np.arange(n, dtype=np.int64) - starts[c_sorted]

    packed = np.zeros(N_BUCKETS * CAP, np.int16)
    packed[c_sorted * CAP + ranks] = loc[order]
    # wrap each sub-op block independently: slot r -> [r % 16, r // 16]
    bounds = np.concatenate([[0], np.cumsum(SUBS)])  # within-bucket sub bounds
    cols = []
    pk = packed.reshape(N_BUCKETS, CAP)
    for j, ni in enumerate(SUBS):
        blk = pk[:, bounds[j] : bounds[j + 1]]                 # [B, ni]
        cols.append(blk.reshape(N_BUCKETS, ni // 16, 16).transpose(0, 2, 1))
    # per bucket: concat sub-blocks along the slot axis -> [B, 16, CAP//16]
    wrapped = np.concatenate(cols, axis=2)
    idx16 = np.tile(
        np.ascontiguousarray(
            wrapped.transpose(1, 0, 2).reshape(16, N_BUCKETS * CAP // 16)
        ),
        (8, 1),
    )

    # within bucket: sub-op j (bounds via searchsorted), r = rank - bounds[j];
    # gather slot r -> partition r%128, tile group bounds[j]/128 + r//128;
    # store puts (p, g) at out row c*CAP + p*G + g.
    j = np.searchsorted(bounds, ranks, side="right") - 1
    r = ranks - bounds[j]
    dev_row_sorted = c_sorted * CAP + (r % 128) * G + bounds[j] // 128 + r // 128
    dev_row = np.empty(n, np.int64)
    dev_row[order] = dev_row_sorted
    return idx16, dev_row


def make_in_maps(indices: np.ndarray, weight: np.ndarray):
    """Shard + pack inputs.  Returns (per_core, in_maps, assemble) where
    assemble(per_core_out_list) -> full [*indices.shape, D] f32 output."""
    idx_flat = np.ascontiguousarray(indices.reshape(-1).astype(np.int32))
    if TABLE_DT == "bf16":
        w = np.ascontiguousarray(weight.astype(ml_dtypes.bfloat16))
    else:
        w = np.ascontiguousarray(weight, dtype=np.float32)
    n_idx = idx_flat.shape[0]
    per_core = n_idx // N_CORES
    assert n_idx == per_core * N_CORES

    in_maps = []
    dev_rows = []
    for cid in range(N_CORES):
        idx16, dev_row = _host_pack(idx_flat[cid * per_core : (cid + 1) * per_core])
        in_maps.append({"idx16": idx16, "weight": w})
        dev_rows.append(dev_row)

    def assemble(outs):
        full = np.concatenate(
            [outs[cid][dev_rows[cid]] for cid in range(N_CORES)], axis=0
        ).astype(np.float32)
        return full.reshape(indices.shape + (D,))

    return per_core, in_maps, assemble


def run_sharded(indices: np.ndarray, weight: np.ndarray, trace: bool = False):
    from concourse.bass_utils import run_bass_kernel_spmd

    per_core, in_maps, assemble = make_in_maps(indices, weight)
    nc = _build_bass(per_core, CAP, tuple(SUBS), BUFS, TABLE_DT)
    res = run_bass_kernel_spmd(
        nc, in_maps, core_ids=list(range(N_CORES)), trace=trace
    )
    return assemble([r["out"] for r in res.results]), res


def kernel(indices: np.ndarray, weight: np.ndarray) -> np.ndarray:
    full, _ = run_sharded(indices, weight, trace=False)
    return full
